# revision 18
# baseline (speedup 1.0000x reference)
"""MoE router kernel for Trainium2 (Bass/Tile), 8-core data-parallel.

Computes, for x [N, D], W [E, D], b [E]:
    logits = x @ W.T + b            # [N, E]
    gate   = softmax(logits, -1)
    top_k_vals, top_k_idx = top_k(gate, 2)
returns (top_k_vals [N,2] f32, top_k_idx [N,2] i32, x [N,D] f32)

Sharding: x split along token axis N across 8 cores; W/b replicated.
On-chip per 128-token tile: PE transposes x chunks (f32) -> PSUM,
DVE/ACT copy to SBUF, PE matmul (xT chunks stationary, W^T moving, N=8)
accumulating logits in PSUM (bias folded in via a ones-column chunk),
then softmax + top-2 epilogue on DVE/ACT.

Built on Bacc (not raw Bass): TRN2 instructions carry at most one sync
wait, and Bacc's compile() splits excess waits into InstEventSemaphore.
"""

import sys

sys.path.insert(0, "/opt/trn_rl_repo")

import numpy as np

import concourse.mybir as mybir
from concourse import bacc
from concourse.masks import make_identity
from concourse.tile import TileContext

N, D, E, TOPK = 65536, 2048, 8, 2
NCORES = 8
P = 128  # partitions / tokens per tile
NPC = N // NCORES  # tokens per core
TPC = NPC // P  # 64 tiles per core
DC = D // P  # 16 contraction chunks
F32 = mybir.dt.float32
I32 = mybir.dt.int32
U32 = mybir.dt.uint32


def build_nc(tpc=TPC):
    nc = bacc.Bacc()
    x_d = nc.dram_tensor("x_sh", [tpc * P, D], F32, kind="ExternalInput")
    # wt[p, c, e] = W[e, c*128 + p]; chunk DC is the bias column b/128
    wt_d = nc.dram_tensor("wt", [P, DC + 1, E], F32, kind="ExternalInput")
    vals_d = nc.dram_tensor("vals", [P, tpc, TOPK], F32, kind="ExternalOutput")
    idx_d = nc.dram_tensor("idx", [P, tpc, TOPK], I32, kind="ExternalOutput")

    with TileContext(nc) as tc:
        with (
            tc.tile_pool(name="const", bufs=1) as cpool,
            tc.tile_pool(name="xin", bufs=4) as xpool,
            tc.tile_pool(name="xt", bufs=2) as xtpool,
            tc.tile_pool(name="ps", bufs=5, space="PSUM") as pspool,
            tc.tile_pool(name="lps", bufs=2, space="PSUM") as lpool,
            tc.tile_pool(name="ep", bufs=3) as epool,
        ):
            ident = cpool.tile([P, P], F32)
            make_identity(nc, ident)
            ones = cpool.tile([P, P], F32)
            nc.gpsimd.memset(ones, 1.0)
            wt_sb = cpool.tile([P, DC + 1, E], F32)
            nc.sync.dma_start(out=wt_sb, in_=wt_d[:])
            sv = cpool.tile([P, tpc, TOPK], F32)
            si = cpool.tile([P, tpc, TOPK], I32)

            for t in range(tpc):
                x_sb = xpool.tile([P, D], F32)
                nc.sync.dma_start(out=x_sb, in_=x_d[t * P : (t + 1) * P, :])

                xT_sb = xtpool.tile([P, D], F32)
                for g in range(4):
                    xT_ps = pspool.tile([P, 4 * P], F32, tag="xT_ps")
                    for cc in range(4):
                        c = 4 * g + cc
                        nc.tensor.transpose(
                            xT_ps[:, cc * P : (cc + 1) * P],
                            x_sb[:, c * P : (c + 1) * P],
                            ident,
                        )
                    dst = xT_sb[:, g * 4 * P : (g + 1) * 4 * P]
                    if t % 2 == 0:
                        nc.vector.tensor_copy(dst, xT_ps)
                    else:
                        nc.scalar.copy(dst, xT_ps)

                lg_ps = lpool.tile([P, E], F32)
                for c in range(DC + 1):
                    nc.tensor.matmul(
                        lg_ps,
                        ones if c == DC else xT_sb[:, c * P : (c + 1) * P],
                        wt_sb[:, c, :],
                        start=(c == 0),
                        stop=(c == DC),
                    )

                # epilogue: top-8+indices on logits, softmax vals for top-2
                lg_sb = epool.tile([P, E], F32)
                if t % 2 == 0:
                    nc.vector.tensor_copy(lg_sb, lg_ps)
                else:
                    nc.scalar.copy(lg_sb, lg_ps)
                mx8 = epool.tile([P, 8], F32)
                nc.vector.max(out=mx8, in_=lg_sb)
                ix8 = epool.tile([P, 8], U32)
                nc.vector.max_index(ix8, mx8, lg_sb)
                ngm = epool.tile([P, 1], F32)
                nc.vector.tensor_scalar_mul(ngm, mx8[:, 0:1], -1.0)
                esb = epool.tile([P, E], F32)
                ssb = epool.tile([P, 1], F32)
                nc.scalar.activation(
                    esb,
                    lg_sb,
                    mybir.ActivationFunctionType.Exp,
                    bias=ngm,
                    scale=1.0,
                    accum_out=ssb,
                )
                rsb = epool.tile([P, 1], F32)
                nc.vector.reciprocal(rsb, ssb)
                ev2 = epool.tile([P, TOPK], F32)
                nc.scalar.activation(
                    ev2,
                    mx8[:, 0:TOPK],
                    mybir.ActivationFunctionType.Exp,
                    bias=ngm,
                    scale=1.0,
                )
                nc.vector.tensor_scalar_mul(sv[:, t, :], ev2, rsb)
                nc.vector.tensor_copy(si[:, t, :], ix8[:, 0:TOPK])

            nc.sync.dma_start(out=vals_d[:], in_=sv)
            nc.sync.dma_start(out=idx_d[:], in_=si)
    nc.finalize()
    return nc


_nc_cache = {}


def get_nc(tpc=TPC):
    if tpc not in _nc_cache:
        _nc_cache[tpc] = build_nc(tpc)
    return _nc_cache[tpc]


def make_in_maps(x, W, b):
    x = np.ascontiguousarray(np.asarray(x, dtype=np.float32))
    W = np.asarray(W, dtype=np.float32)
    b = np.asarray(b, dtype=np.float32)
    # wt[p, c, e] = W[e, c*128 + p] for c < DC; wt[p, DC, e] = b[e] / 128
    wt = np.empty((P, DC + 1, E), dtype=np.float32)
    wt[:, :DC, :] = W.T.reshape(DC, P, E).transpose(1, 0, 2)
    wt[:, DC, :] = b.reshape(1, E) / P
    return [
        {"x_sh": x[c * NPC : (c + 1) * NPC], "wt": wt} for c in range(NCORES)
    ], x


def unshard(results):
    vals = np.concatenate(
        [r["vals"].transpose(1, 0, 2).reshape(NPC, TOPK) for r in results], axis=0
    )
    idx = np.concatenate(
        [r["idx"].transpose(1, 0, 2).reshape(NPC, TOPK) for r in results], axis=0
    )
    return vals, idx.astype(np.int32)


def run(x, W, b, trace=False, **kwargs):
    from concourse.bass_utils import run_bass_kernel_spmd

    in_maps, x_np = make_in_maps(x, W, b)
    res = run_bass_kernel_spmd(
        get_nc(), in_maps, core_ids=list(range(NCORES)), trace=trace, **kwargs
    )
    vals, idx = unshard(res.results)
    return (vals, idx, x_np), res


def kernel(x, W, b):
    (vals, idx, x_np), _ = run(x, W, b, trace=False)
    return vals, idx, x_np
